# revision 2
# baseline (speedup 1.0000x reference)
"""3-layer GAT on Trainium2, 8 NeuronCores.

Strategy (dst-sharded, slot-major, identity-matmul aggregation):
- Nodes dst-sharded across 8 cores (12500/core). Within a shard, dsts are
  permuted by in-degree so each block of 128 dsts has near-uniform degree.
- Edges of a block are laid out slot-major: slot j holds the j-th incoming
  edge of each of the 128 dsts (padded to the block max degree with a
  poisoned table row: h=0, a_src-logit=-60 => alpha ~ 0).
- Per-layer node feature tables T_l hold fp16 rows [h_l | as_l] so a single
  indirect row gather per block delivers both the features and the source
  attention logits. The dst logits come from a small resident per-shard
  table. Softmax is computed without the segment max (logits are O(1)).
- The scatter-add becomes PSUM accumulation of per-slot [128, F] tiles via
  matmuls with a stationary identity matrix.
- Layer outputs never leave the core: each block's epilogue (LN+ELU) feeds
  the next layer's table-row build (h_{l+1} = act @ W_{l+1}) on the spot.
  Full tables are assembled on the host between the 4 SPMD launches
  (collectives are not available in this runtime).
"""

import numpy as np

import concourse.bass as bass
import concourse.tile as tile
from concourse import bacc, mybir
from concourse.bass_utils import run_bass_kernel_spmd
from contextlib import ExitStack

F16 = mybir.dt.float16
F32 = mybir.dt.float32
I32 = mybir.dt.int32

CORES = 8
N = 100000
SH = 12500          # real dsts per core
SHP = 12544         # padded (98 * 128)
NB = SHP // 128     # 98 blocks
TBL_ROWS = CORES * SHP + 128   # + pad block
PAD_ROW = CORES * SHP
NEG = 0.2
EPS = 1e-5
PAD_AS = -60.0

# per-layer configs
L1 = dict(H=8, FT=128, TROW=136, ASOFF=128, TAILN=80, KW=128)
L2 = dict(H=4, FT=64, TROW=72, ASOFF=64, TAILN=32, KW=64)
L3 = dict(H=1, FT=16, TROW=16, ASOFF=10, TAILN=0, KW=0)


def _host_prep(edge_index):
    """Static index prep from edge_index only (graph layout, no NN compute)."""
    src = np.concatenate([edge_index[0].astype(np.int64), np.arange(N, dtype=np.int64)])
    dst = np.concatenate([edge_index[1].astype(np.int64), np.arange(N, dtype=np.int64)])
    deg = np.bincount(dst, minlength=N)

    row_of = np.empty(N, dtype=np.int64)
    order = np.argsort(dst, kind="stable")
    src_by_dst = src[order]
    starts = np.zeros(N + 1, dtype=np.int64)
    np.cumsum(deg, out=starts[1:])

    per_core = []
    for c in range(CORES):
        nodes = np.arange(c * SH, (c + 1) * SH)
        perm = np.argsort(deg[nodes], kind="stable")  # ascending degree
        perm_nodes = nodes[perm]
        npad = SHP - SH
        pos_node = np.full(SHP, -1, dtype=np.int64)
        pos_node[npad:] = perm_nodes
        row_of[perm_nodes] = c * SHP + npad + np.arange(SH)
        per_core.append(dict(pos_node=pos_node))

    # common per-block slot counts (max across cores)
    S_list = np.ones(NB, dtype=np.int64)
    for c in range(CORES):
        pos_node = per_core[c]["pos_node"]
        for b in range(NB):
            blk = pos_node[b * 128 : (b + 1) * 128]
            m = max((int(deg[n]) for n in blk if n >= 0), default=1)
            S_list[b] = max(S_list[b], max(1, m))
    offs = np.concatenate([[0], np.cumsum(S_list)]).astype(np.int64)
    totS = int(offs[-1])

    for c in range(CORES):
        pos_node = per_core[c]["pos_node"]
        idx_pm = np.full((128, totS), PAD_ROW, dtype=np.int32)
        for b in range(NB):
            o = offs[b]
            blk = pos_node[b * 128 : (b + 1) * 128]
            for p, n in enumerate(blk):
                if n < 0:
                    continue
                es = np.sort(src_by_dst[starts[n] : starts[n + 1]])
                idx_pm[p, o : o + len(es)] = row_of[es].astype(np.int32)
        per_core[c]["idx_pm"] = np.ascontiguousarray(idx_pm)
    return per_core, S_list, totS


def _wt_tables(inputs):
    W1, W2, W3 = (np.asarray(inputs["W1"], np.float32), np.asarray(inputs["W2"], np.float32),
                  np.asarray(inputs["W3"], np.float32))

    def fold(a):
        a = np.asarray(a, np.float32)
        H, F = a.shape
        m = np.zeros((H * F, H), np.float32)
        for k in range(H):
            m[k * F : (k + 1) * F, k] = a[k]
        return m

    wt1 = np.concatenate([W1, W1 @ fold(inputs["a_src1"]), W1 @ fold(inputs["a_dst1"])], axis=1)
    wt2 = np.concatenate(
        [W2, W2 @ fold(inputs["a_src2"]), np.zeros((128, 4), np.float32),
         W2 @ fold(inputs["a_dst2"]), np.zeros((128, 4), np.float32)], axis=1)
    wt3 = np.concatenate(
        [W3, W3 @ fold(inputs["a_src3"]), np.zeros((64, 5), np.float32),
         W3 @ fold(inputs["a_dst3"]), np.zeros((64, 15), np.float32)], axis=1)
    return wt1.astype(np.float16), wt2.astype(np.float16), wt3.astype(np.float16)


def _build_launch0():
    nc = bacc.Bacc("TRN2", target_bir_lowering=False, debug=False, num_devices=CORES)
    x_t = nc.dram_tensor("x_t", [128, SHP], F16, kind="ExternalInput")
    wt1 = nc.dram_tensor("wt1", [128, 144], F16, kind="ExternalInput")
    tb = nc.dram_tensor("tb", [SHP, 144], F16, kind="ExternalOutput")
    with tile.TileContext(nc) as tc, ExitStack() as ctx:
        cpool = ctx.enter_context(tc.tile_pool(name="c", bufs=1))
        epool = ctx.enter_context(tc.tile_pool(name="e", bufs=3))
        pp = ctx.enter_context(tc.tile_pool(name="pp", bufs=2, space="PSUM"))
        xs = cpool.tile([128, SHP], F16)
        nc.sync.dma_start(xs[:, :], x_t[:, :])
        ws = cpool.tile([128, 144], F16)
        nc.sync.dma_start(ws[:, :], wt1[:, :])
        for b in range(NB):
            ps = pp.tile([128, 144], F32, tag="ps")
            nc.tensor.matmul(ps[:, :], xs[:, b * 128 : (b + 1) * 128], ws[:, :],
                             start=True, stop=True)
            ev = epool.tile([128, 144], F16, tag="ev")
            nc.vector.tensor_copy(ev[:, :], ps[:, :])
            nc.sync.dma_start(tb[b * 128 : (b + 1) * 128, :], ev[:, :])
    nc.compile()
    return nc


def _build_agg(cfg, S_list, layer):
    TROW, H, FT, ASOFF, TAILN, KW = (cfg["TROW"], cfg["H"], cfg["FT"], cfg["ASOFF"],
                                     cfg["TAILN"], cfg["KW"])
    totS = int(np.sum(S_list))
    Smax = int(np.max(S_list))

    nc = bacc.Bacc("TRN2", target_bir_lowering=False, debug=False, num_devices=CORES)
    T = nc.dram_tensor("T", [TBL_ROWS, TROW], F16, kind="ExternalInput")
    idx_d = nc.dram_tensor("idx", [128, totS], I32, kind="ExternalInput")
    ad_d = nc.dram_tensor("ad", [128, NB * H], F16, kind="ExternalInput")
    ident_d = nc.dram_tensor("ident", [128, 128], F16, kind="ExternalInput")
    if layer < 3:
        wt_d = nc.dram_tensor("wt", [KW, TAILN], F16, kind="ExternalInput")
        out_d = nc.dram_tensor("out", [SHP, TAILN], F16, kind="ExternalOutput")
    else:
        out_d = nc.dram_tensor("out", [SHP, 10], F32, kind="ExternalOutput")

    with tile.TileContext(nc) as tc, ExitStack() as ctx:
        cpool = ctx.enter_context(tc.tile_pool(name="c", bufs=1))
        gpool = ctx.enter_context(tc.tile_pool(name="g", bufs=3))
        apool = ctx.enter_context(tc.tile_pool(name="a", bufs=3))
        epool = ctx.enter_context(tc.tile_pool(name="e", bufs=3))
        ppo = ctx.enter_context(tc.tile_pool(name="ppo", bufs=2, space="PSUM"))
        ppt = ctx.enter_context(tc.tile_pool(name="ppt", bufs=2, space="PSUM"))

        idx_sb = cpool.tile([128, totS], I32)
        nc.sync.dma_start(idx_sb[:, :], idx_d[:, :])
        ad_sb = cpool.tile([128, NB * H], F16)
        nc.sync.dma_start(ad_sb[:, :], ad_d[:, :])
        ident = cpool.tile([128, 128], F16)
        nc.sync.dma_start(ident[:, :], ident_d[:, :])
        if layer < 3:
            wt_sb = cpool.tile([KW, TAILN], F16)
            nc.sync.dma_start(wt_sb[:, :], wt_d[:, :])
        else:
            ls_sb = cpool.tile([128, NB, 16], F32)

        off = 0
        for b in range(NB):
            S = int(S_list[b])
            G = gpool.tile([128, Smax, TROW], F16, tag="G")
            # one gathered row per partition per instruction: multi-offset
            # indirect DMA is not supported by this runtime's DGE config
            for j in range(S):
                nc.gpsimd.indirect_dma_start(
                    out=G[:, j, :], out_offset=None, in_=T[:, :],
                    in_offset=bass.IndirectOffsetOnAxis(
                        ap=idx_sb[:, off + j : off + j + 1], axis=0))

            z = apool.tile([128, H, Smax], F32, tag="z")
            nc.vector.tensor_tensor(
                out=z[:, :, :S],
                in0=G[:, :S, ASOFF : ASOFF + H].rearrange("p s h -> p h s"),
                in1=ad_sb[:, b * H : (b + 1) * H].to_broadcast([128, H, S]),
                op=mybir.AluOpType.add)
            # exp(leaky_relu(z)) == max(exp(z), exp(NEG*z))
            e_a = apool.tile([128, H, Smax], F16, tag="e_a")
            nc.scalar.activation(e_a[:, :, :S], z[:, :, :S],
                                 mybir.ActivationFunctionType.Exp)
            e_b = apool.tile([128, H, Smax], F16, tag="e_b")
            nc.scalar.activation(e_b[:, :, :S], z[:, :, :S],
                                 mybir.ActivationFunctionType.Exp, scale=NEG)
            al = apool.tile([128, H, Smax], F16, tag="al")
            nc.vector.tensor_tensor(out=al[:, :, :S], in0=e_a[:, :, :S],
                                    in1=e_b[:, :, :S], op=mybir.AluOpType.max)
            s_t = apool.tile([128, H], F32, tag="s")
            nc.vector.tensor_reduce(s_t[:, :], al[:, :, :S], axis=mybir.AxisListType.X,
                                    op=mybir.AluOpType.add)
            rcp = apool.tile([128, H], F32, tag="rcp")
            nc.vector.reciprocal(rcp[:, :], s_t[:, :])
            ah = apool.tile([128, H, Smax], F16, tag="ah")
            nc.vector.tensor_tensor(out=ah[:, :, :S], in0=al[:, :, :S],
                                    in1=rcp[:, :].to_broadcast([128, H, S]),
                                    op=mybir.AluOpType.mult)
            ax = apool.tile([128, Smax, FT], F16, tag="ax")
            nc.scalar.activation(
                ax[:, :S, :].rearrange("p s (h r) -> p s h r", h=H),
                ah[:, :, :S].rearrange("p h s -> p s h").to_broadcast([128, S, H, FT // H]),
                mybir.ActivationFunctionType.Copy)
            Gp = gpool.tile([128, Smax, FT], F16, tag="Gp")
            nc.vector.tensor_tensor(out=Gp[:, :S, :], in0=G[:, :S, 0:FT], in1=ax[:, :S, :],
                                    op=mybir.AluOpType.mult)
            po = ppo.tile([128, FT], F32, tag="po")
            for j in range(S):
                nc.tensor.matmul(po[:, :], ident[:, :], Gp[:, j, :],
                                 start=(j == 0), stop=(j == S - 1))

            if layer < 3:
                bn6 = epool.tile([128, 6], F32, tag="bn6")
                nc.vector.bn_stats(bn6[:, :], po[:, :])
                mv = epool.tile([128, 2], F32, tag="mv")
                nc.vector.bn_aggr(mv[:, :], bn6[:, :])
                vpe = epool.tile([128, 1], F32, tag="vpe")
                nc.vector.tensor_scalar(out=vpe[:, :], in0=mv[:, 1:2], scalar1=EPS,
                                        scalar2=None, op0=mybir.AluOpType.add)
                sd = epool.tile([128, 1], F32, tag="sd")
                nc.scalar.activation(sd[:, :], vpe[:, :],
                                     mybir.ActivationFunctionType.Sqrt)
                rstd = epool.tile([128, 1], F32, tag="rstd")
                nc.vector.reciprocal(rstd[:, :], sd[:, :])
                y = epool.tile([128, FT], F32, tag="y")
                nc.vector.tensor_scalar(out=y[:, :], in0=po[:, :], scalar1=mv[:, 0:1],
                                        scalar2=rstd[:, :], op0=mybir.AluOpType.subtract,
                                        op1=mybir.AluOpType.mult)
                ym = epool.tile([128, FT], F32, tag="ym")
                nc.vector.tensor_scalar(out=ym[:, :], in0=y[:, :], scalar1=0.0,
                                        scalar2=None, op0=mybir.AluOpType.min)
                ee = epool.tile([128, FT], F32, tag="ee")
                nc.scalar.activation(ee[:, :], ym[:, :], mybir.ActivationFunctionType.Exp)
                e1 = epool.tile([128, FT], F32, tag="e1")
                nc.vector.tensor_scalar(out=e1[:, :], in0=ee[:, :], scalar1=1.0,
                                        scalar2=None, op0=mybir.AluOpType.subtract)
                act = epool.tile([128, FT], F16, tag="act")
                nc.vector.tensor_tensor(out=act[:, :], in0=y[:, :], in1=e1[:, :],
                                        op=mybir.AluOpType.max)
                pt = ppt.tile([FT, 128], F16, tag="pt")
                nc.tensor.transpose(pt[:, :], act[:, :], ident[:, :])
                at = epool.tile([FT, 128], F16, tag="at")
                nc.vector.tensor_copy(at[:, :], pt[:, :])
                p2 = ppt.tile([128, TAILN], F32, tag="p2")
                nc.tensor.matmul(p2[:, :], at[:, :], wt_sb[:, :], start=True, stop=True)
                ev = epool.tile([128, TAILN], F16, tag="ev")
                nc.vector.tensor_copy(ev[:, :], p2[:, :])
                nc.sync.dma_start(out_d[b * 128 : (b + 1) * 128, :], ev[:, :])
            else:
                nc.vector.tensor_copy(ls_sb[:, b, 0:10], po[:, 0:10])
            off += S

        if layer == 3:
            rmax = cpool.tile([128, NB], F32)
            nc.vector.tensor_reduce(rmax[:, :], ls_sb[:, :, 0:10], axis=mybir.AxisListType.X,
                                    op=mybir.AluOpType.max)
            tt = cpool.tile([128, NB, 10], F32)
            nc.vector.tensor_tensor(out=tt[:, :, :], in0=ls_sb[:, :, 0:10],
                                    in1=rmax[:, :].to_broadcast([128, NB, 10]),
                                    op=mybir.AluOpType.subtract)
            ex = cpool.tile([128, NB, 10], F32)
            nc.scalar.activation(ex[:, :, :], tt[:, :, :], mybir.ActivationFunctionType.Exp)
            ssum = cpool.tile([128, NB], F32)
            nc.vector.tensor_reduce(ssum[:, :], ex[:, :, :], axis=mybir.AxisListType.X,
                                    op=mybir.AluOpType.add)
            lg = cpool.tile([128, NB], F32)
            nc.scalar.activation(lg[:, :], ssum[:, :], mybir.ActivationFunctionType.Ln)
            fin = cpool.tile([128, NB, 10], F32)
            nc.vector.tensor_tensor(out=fin[:, :, :], in0=tt[:, :, :],
                                    in1=lg[:, :].to_broadcast([128, NB, 10]),
                                    op=mybir.AluOpType.subtract)
            nc.sync.dma_start(out_d.ap().rearrange("(b p) c -> p b c", p=128), fin[:, :, :])
    nc.compile()
    return nc


def _ad_layout(tbl, H):
    """[SHP, H] (perm order) -> [128, NB*H] with [p, b*H+k] = tbl[b*128+p, k]."""
    return np.ascontiguousarray(tbl.reshape(NB, 128, H).transpose(1, 0, 2).reshape(128, NB * H))


LAST_EXEC_NS = 0
_TRACE_OK = None


def _trace_available():
    global _TRACE_OK
    if _TRACE_OK is None:
        try:
            from antenv.axon_hooks import get_axon_ntff_profile_hook
            _TRACE_OK = get_axon_ntff_profile_hook() is not None
        except Exception:
            _TRACE_OK = False
    return _TRACE_OK


def _hw_runner(nc, in_maps):
    global LAST_EXEC_NS
    try:
        r = run_bass_kernel_spmd(nc, in_maps, core_ids=list(range(CORES)),
                                 trace=_trace_available())
        if r.exec_time_ns:
            LAST_EXEC_NS += r.exec_time_ns
    except Exception:
        r = run_bass_kernel_spmd(nc, in_maps, core_ids=list(range(CORES)), trace=False)
    return r.results


def kernel(_runner=None, **inputs):
    runner = _runner or _hw_runner
    x = np.asarray(inputs["x"])
    edge_index = np.asarray(inputs["edge_index"])
    pc, S_list, totS = _host_prep(edge_index)
    wt1, wt2, wt3 = _wt_tables(inputs)
    ident = np.eye(128, dtype=np.float16)

    # ---- launch 0: T1 shard build ----
    nc0 = _build_launch0()
    in_maps0 = []
    for c in range(CORES):
        pos_node = pc[c]["pos_node"]
        xs = np.zeros((SHP, 128), np.float16)
        real = pos_node >= 0
        xs[real] = x[pos_node[real]].astype(np.float16)
        in_maps0.append({"x_t": np.ascontiguousarray(xs.T), "wt1": wt1})
    r0 = runner(nc0, in_maps0)
    tb = [r0[c]["tb"] for c in range(CORES)]

    T = np.zeros((TBL_ROWS, 136), np.float16)
    for c in range(CORES):
        T[c * SHP : (c + 1) * SHP] = tb[c][:, 0:136]
    T[PAD_ROW:, 128:136] = PAD_AS
    ads = [_ad_layout(tb[c][:, 136:144], 8) for c in range(CORES)]

    for layer, cfg in ((1, L1), (2, L2), (3, L3)):
        nc = _build_agg(cfg, S_list, layer)
        in_maps = []
        for c in range(CORES):
            m = {"T": T, "idx": pc[c]["idx_pm"], "ad": ads[c], "ident": ident}
            if layer == 1:
                m["wt"] = wt2
            elif layer == 2:
                m["wt"] = wt3
            in_maps.append(m)
        r = runner(nc, in_maps)
        outs = [r[c]["out"] for c in range(CORES)]
        if layer == 1:
            T = np.zeros((TBL_ROWS, 72), np.float16)
            for c in range(CORES):
                T[c * SHP : (c + 1) * SHP] = outs[c][:, 0:72]
            T[PAD_ROW:, 64:68] = PAD_AS
            ads = [_ad_layout(outs[c][:, 72:76], 4) for c in range(CORES)]
        elif layer == 2:
            T = np.zeros((TBL_ROWS, 16), np.float16)
            for c in range(CORES):
                T[c * SHP : (c + 1) * SHP] = outs[c][:, 0:16]
            T[PAD_ROW:, 10] = PAD_AS
            ads = [_ad_layout(outs[c][:, 16:17], 1) for c in range(CORES)]

    # un-permute final outputs
    result = np.empty((N, 10), np.float32)
    for c in range(CORES):
        pos_node = pc[c]["pos_node"]
        real = pos_node >= 0
        result[pos_node[real]] = outs[c][real]
    return result



# revision 19
# speedup vs baseline: 3.5269x; 3.5269x over previous
"""3-layer GAT on Trainium2, 8 NeuronCores.

Strategy (dst-sharded, slot-major, dma_gather + identity-matmul aggregation):
- Nodes dst-sharded across 8 cores. Within a shard, dsts are permuted by
  (degree, per-window degree profile) so blocks of 128 dsts have uniform
  slot needs. Sources are partitioned into 4 table windows of <=32768 rows
  (int16 dma_gather index limit); a host greedy balances per-dst window
  in-degrees and a repair pass duplicates src rows into spare window
  capacity so the shared per-(group, window) slot counts C[g, w] approach
  the ceil(dmax/4) floor.
- Table rows hold [h | exp(as) | exp(0.2*as)] in f16. By softmax scale
  invariance the dst logit reduces to one multiply with
  gamma = exp(-0.8*ad): w_e = max(A, A2*gamma) — no per-edge exps.
- Gathers run in <=1024-index dma_gather chunks (HW ucode limit), streamed
  (batch-of-6-groups, window)-major so the per-group PSUM accumulators
  stay live across all 4 windows (6 po banks + 2 epilogue banks).
- Weights are applied UNNORMALIZED via an alpha-expand (Act/DVE split) and
  one f16 multiply; slot tiles accumulate per group-pair with identity
  matmuls ([128, 2*FT] moving operand covers both blocks). Normalization
  by the softmax denominator happens once per dst in the epilogue.
- Epilogue: LN (rstd via Ln+Exp so the Act table never swaps), ELU, next
  layer's row build (act @ W_{l+1} with doubled a_src folds, then exp on
  the logit columns). Tables are assembled on the host between launches.
"""

import numpy as np

import concourse.bass as bass
import concourse.tile as tile
from concourse import bacc, mybir
from concourse.bass_utils import run_bass_kernel_spmd
from contextlib import ExitStack

F16 = mybir.dt.float16
F32 = mybir.dt.float32
I16 = mybir.dt.int16

CORES = 8
N = 100000
SH = 12500          # real dsts per core
SHP = 12544         # padded (98 * 128)
NB = SHP // 128     # 98 blocks
NG = NB // 2        # 49 groups of 2 blocks
GBATCH = 3          # groups per PSUM-resident batch
NW = 4              # gather windows
WIN = 32768         # rows per window
PAD_LOCAL = WIN - 1 # per-window pad row (all zero => zero weight)
WCAP = 32700        # window capacity for node assignment
EPS = 1e-5
SCAP = 48           # superchunk staging capacity in slots
GMAX = 8            # max slots per dma_gather (1024 idx HW limit)

# ELEM = gathered row f16 elements (256B multiple); AOFF/A2OFF = col of
# exp(as)/exp(0.2 as); TAILN = epilogue output cols; KW = wt rows (=FT)
L1 = dict(H=8, FT=128, ELEM=256, AOFF=128, A2OFF=136, TAILN=76, KW=128,
          WOUT=64, HN=4)
L2 = dict(H=4, FT=64, ELEM=128, AOFF=64, A2OFF=68, TAILN=16, KW=64,
          WOUT=10, HN=1)
L3 = dict(H=1, FT=16, ELEM=128, AOFF=10, A2OFF=11, TAILN=0, KW=0,
          WOUT=0, HN=0)


# ---------------------------------------------------------------- host prep

def _window_assign(src, dst, rounds=2, tighten=3, seed=1):
    """Greedy window assignment balancing per-dst window in-degrees, then
    tightening rounds targeting the actual shared per-(group, window)
    slot maxima."""
    deg = np.bincount(dst, minlength=N)
    order_src = np.argsort(src, kind="stable")
    dst_by_src = dst[order_src]
    starts = np.zeros(N + 1, np.int64)
    np.cumsum(np.bincount(src, minlength=N), out=starts[1:])

    w_of = np.full(N, -1, np.int32)
    k = np.zeros((N, NW), np.int32)
    cnt = np.zeros(NW, np.int64)
    order = np.random.default_rng(seed).permutation(N)
    quota = ((deg + NW - 1) // NW)[:, None] + np.zeros((1, NW), np.int64)

    def greedy_round(cap):
        for u in order:
            vs = dst_by_src[starts[u]:starts[u + 1]]
            if w_of[u] >= 0:
                k[vs, w_of[u]] -= 1
                cnt[w_of[u]] -= 1
            kv = k[vs]
            over = np.maximum(kv + 1 - cap[vs], 0)
            costs = (over ** 2).sum(axis=0) * 100.0 + kv.sum(axis=0) + cnt / 500.0
            costs[cnt >= WCAP] = np.inf
            w = int(np.argmin(costs))
            w_of[u] = w
            k[vs, w] += 1
            cnt[w] += 1

    def block_C():
        g_of = np.zeros(N, np.int64)
        C = np.zeros((NG, NW), np.int64)
        kful = np.zeros((SHP, NW), np.int64)
        npad = SHP - SH
        for c in range(CORES):
            nodes = np.arange(c * SH, (c + 1) * SH)
            keys = tuple(k[nodes, j] for j in range(NW - 1, -1, -1)) + (deg[nodes],)
            perm = nodes[np.lexsort(keys)]
            g_of[perm] = (npad + np.arange(SH)) // 256
            kful[:] = 0
            kful[npad:] = k[perm]
            C = np.maximum(C, kful.reshape(NG, 256, NW).max(axis=1))
        return np.maximum(C, 1), g_of

    for _ in range(rounds):
        greedy_round(quota)
    for _ in range(tighten):
        C, g_of = block_C()
        cap = np.maximum(C - 1, 1)[g_of]
        greedy_round(np.maximum(cap, quota))
    return w_of, k, deg


def _layout(C):
    """Slot layout: batches of GBATCH groups; within a batch, windows in
    order; within (batch, w), the batch's groups block-major. Superchunks
    pack whole (g, w) segments up to SCAP slots.

    Returns (seg_off[g, w] absolute slot offsets, tot_slots, plan) where
    plan is a list of batches; each batch is a list over windows of lists
    of superchunks; each superchunk is (start_slot, [(g, off_in_chunk, Cg)]).
    """
    seg_off = np.zeros((NG, NW), np.int64)
    plan = []
    pos = 0
    for b0 in range(0, NG, GBATCH):
        groups = list(range(b0, min(b0 + GBATCH, NG)))
        wplans = []
        for w in range(NW):
            chunks, cur, coff = [], [], 0
            cstart = pos
            for g in groups:
                need = 2 * int(C[g, w])
                if coff + need > SCAP and cur:
                    chunks.append((cstart, cur))
                    cstart, cur, coff = pos, [], 0
                seg_off[g, w] = pos
                cur.append((g, coff, int(C[g, w])))
                coff += need
                pos += need
            if cur:
                chunks.append((cstart, cur))
            wplans.append(chunks)
        plan.append((groups, wplans))
    return seg_off, int(pos), plan


def _host_prep(edge_index):
    """Graph-structure prep (host): window assignment, per-core dst perm,
    per-edge window repair via source-row duplication, shared slot counts,
    and per-core wrapped-int16 gather index streams."""
    src = np.concatenate([edge_index[0].astype(np.int64), np.arange(N, dtype=np.int64)])
    dst = np.concatenate([edge_index[1].astype(np.int64), np.arange(N, dtype=np.int64)])
    w_of, k, deg = _window_assign(src, dst)
    npad = SHP - SH

    per_core = []
    g_of = np.zeros(N, np.int64)
    dmaxg = np.zeros(NG, np.int64)
    for c in range(CORES):
        nodes = np.arange(c * SH, (c + 1) * SH)
        keys = tuple(k[nodes, j] for j in range(NW - 1, -1, -1)) + (deg[nodes],)
        perm = nodes[np.lexsort(keys)]
        pos_node = np.full(SHP, -1, np.int64)
        pos_node[npad:] = perm
        per_core.append(dict(pos_node=pos_node))
        g_of[perm] = (npad + np.arange(SH)) // 256
        dful = np.zeros(SHP, np.int64)
        dful[npad:] = deg[perm]
        dmaxg = np.maximum(dmaxg, dful.reshape(NG, 256).max(axis=1))

    target = np.maximum((dmaxg + NW - 1) // NW, 1)

    # per-edge window repair by duplicating src rows into spare capacity
    EE = len(src)
    e_win = w_of[src].astype(np.int32)
    kk = k.astype(np.int64).copy()
    od = np.lexsort((src, dst))
    dstart = np.zeros(N + 1, np.int64)
    np.cumsum(np.bincount(dst, minlength=N), out=dstart[1:])
    wcount = np.bincount(w_of, minlength=NW).astype(np.int64)
    tgt_v = target[g_of]
    sod = np.argsort(src, kind="stable")
    sstart = np.zeros(N + 1, np.int64)
    np.cumsum(np.bincount(src, minlength=N), out=sstart[1:])
    copy_key = {}
    for u in range(N):
        copy_key[(u, int(w_of[u]))] = True
    for g in range(NG):
        tgt = int(target[g])
        vs = np.where(g_of == g)[0]
        for w in range(NW):
            bad = vs[kk[vs, w] > tgt]
            for v in bad:
                es = od[dstart[v]:dstart[v + 1]]
                need = int(kk[v, w] - tgt)
                cand = [e for e in es if e_win[e] == w]
                for e in cand:
                    if need == 0:
                        break
                    u = int(src[e])
                    best, best_new = -1, -1
                    for wp in np.argsort(kk[v]):
                        wp = int(wp)
                        if wp == w or kk[v, wp] >= tgt:
                            continue
                        if (u, wp) in copy_key:
                            best = wp
                            break
                        if best_new < 0 and wcount[wp] < WIN - 2:
                            best_new = wp
                    created = False
                    if best < 0:
                        if best_new < 0:
                            continue
                        best = best_new
                        copy_key[(u, best)] = True
                        wcount[best] += 1
                        created = True
                    e_win[e] = best
                    kk[v, w] -= 1
                    kk[v, best] += 1
                    need -= 1
                    if created:
                        # amortize the new copy: move u's other violated
                        # edges into this window where their dsts have room
                        for e2 in sod[sstart[u]:sstart[u + 1]]:
                            w2 = int(e_win[e2])
                            if w2 == best:
                                continue
                            v2 = int(dst[e2])
                            t2 = int(tgt_v[v2])
                            if kk[v2, w2] > t2 and kk[v2, best] < t2:
                                e_win[e2] = best
                                kk[v2, w2] -= 1
                                kk[v2, best] += 1

    C = np.zeros((NG, NW), np.int64)
    for c in range(CORES):
        pos_node = per_core[c]["pos_node"]
        kful = np.zeros((SHP, NW), np.int64)
        kful[npad:] = kk[pos_node[npad:]]
        C = np.maximum(C, kful.reshape(NG, 256, NW).max(axis=1))
    C = np.maximum(C, 1)
    assert int(C.max()) * 2 <= SCAP

    # local rows for every (node, window) copy
    rows_node = np.fromiter((p[0] for p in copy_key), np.int64, len(copy_key))
    rows_win = np.fromiter((p[1] for p in copy_key), np.int64, len(copy_key))
    rorder = np.lexsort((rows_node, rows_win))
    rows_node, rows_win = rows_node[rorder], rows_win[rorder]
    rows_local = np.zeros(len(rows_node), np.int64)
    for w in range(NW):
        m = rows_win == w
        assert m.sum() < WIN - 1
        rows_local[m] = np.arange(m.sum())
    local_lut = {}
    for u, w, l in zip(rows_node, rows_win, rows_local):
        local_lut[(int(u), int(w))] = int(l)
    e_local = np.fromiter(
        (local_lut[(int(u), int(w))] for u, w in zip(src, e_win)),
        np.int64, EE)
    rows_global = rows_win * WIN + rows_local

    seg_off, tot_slots, _plan = _layout(C)

    order = np.lexsort((e_local, e_win, dst))
    e_dst2, e_w2, e_l2 = dst[order], e_win[order], e_local[order]
    key = e_dst2 * NW + e_w2
    newrun = np.ones(len(key), bool)
    newrun[1:] = key[1:] != key[:-1]
    run_start = np.maximum.accumulate(np.where(newrun, np.arange(len(key)), 0))
    e_rank = np.arange(len(key)) - run_start

    for c in range(CORES):
        pos_node = per_core[c]["pos_node"]
        pos_of = np.full(N, -1, np.int64)
        real = pos_node >= 0
        pos_of[pos_node[real]] = np.nonzero(real)[0]
        sel = (e_dst2 >= c * SH) & (e_dst2 < (c + 1) * SH)
        v_pos = pos_of[e_dst2[sel]]
        w_e = e_w2[sel]
        bb = v_pos // 128
        p = v_pos % 128
        g = bb // 2
        b = bb % 2
        Cg = C[g, w_e]
        flat = ((seg_off[g, w_e] + b * Cg + e_rank[sel]) * 128 + p)
        stream = np.full(tot_slots * 128, PAD_LOCAL, np.int32)
        stream[flat] = e_l2[sel]
        wrapped = stream.reshape(-1, 16).T.astype(np.uint16).view(np.int16)
        per_core[c]["idx"] = np.ascontiguousarray(np.tile(wrapped, (8, 1)))
    return per_core, C, (rows_global, rows_node), tot_slots


def _wt_tables(inputs):
    W1, W2, W3 = (np.asarray(inputs["W1"], np.float32), np.asarray(inputs["W2"], np.float32),
                  np.asarray(inputs["W3"], np.float32))

    def fold(a):
        a = np.asarray(a, np.float32)
        H, F = a.shape
        m = np.zeros((H * F, H), np.float32)
        for kk in range(H):
            m[kk * F : (kk + 1) * F, kk] = a[kk]
        return m

    # [W | as-fold | as-fold | ad-fold]: the two as copies become
    # exp(as) and exp(0.2 as) in the epilogue
    s1, d1 = W1 @ fold(inputs["a_src1"]), W1 @ fold(inputs["a_dst1"])
    wt1 = np.concatenate([W1, s1, s1, d1], axis=1)                     # [128,152]
    s2, d2 = W2 @ fold(inputs["a_src2"]), W2 @ fold(inputs["a_dst2"])
    wt2 = np.concatenate([W2, s2, s2, d2], axis=1)                     # [128,76]
    s3, d3 = W3 @ fold(inputs["a_src3"]), W3 @ fold(inputs["a_dst3"])
    wt3 = np.concatenate([W3, s3, s3, d3, np.zeros((64, 3), np.float32)],
                         axis=1)                                       # [64,16]
    return wt1.astype(np.float16), wt2.astype(np.float16), wt3.astype(np.float16)


# ---------------------------------------------------------------- launches

def _build_launch0():
    nc = bacc.Bacc("TRN2", target_bir_lowering=False, debug=False, num_devices=CORES)
    x_t = nc.dram_tensor("x_t", [128, SHP], F16, kind="ExternalInput")
    wt1 = nc.dram_tensor("wt1", [128, 152], F16, kind="ExternalInput")
    tb = nc.dram_tensor("tb", [SHP, 152], F16, kind="ExternalOutput")
    with tile.TileContext(nc) as tc, ExitStack() as ctx:
        cpool = ctx.enter_context(tc.tile_pool(name="c", bufs=1))
        epool = ctx.enter_context(tc.tile_pool(name="e", bufs=3))
        pp = ctx.enter_context(tc.tile_pool(name="pp", bufs=2, space="PSUM"))
        xs = cpool.tile([128, SHP], F16)
        nc.sync.dma_start(xs[:, :], x_t[:, :])
        ws = cpool.tile([128, 152], F16)
        nc.sync.dma_start(ws[:, :], wt1[:, :])
        BB = 7
        for b0 in range(0, NB, BB):
            nb = min(BB, NB - b0)
            ev = epool.tile([128, BB, 152], F16, tag="ev")
            for i in range(nb):
                b = b0 + i
                ps = pp.tile([128, 152], F32, tag="ps")
                nc.tensor.matmul(ps[:, :], xs[:, b * 128 : (b + 1) * 128], ws[:, :],
                                 start=True, stop=True)
                nc.vector.tensor_copy(ev[:, i, :], ps[:, :])
                nc.scalar.activation(ev[:, i, 128:136], ps[:, 128:136],
                                     mybir.ActivationFunctionType.Exp)
                nc.scalar.activation(ev[:, i, 136:144], ps[:, 136:144],
                                     mybir.ActivationFunctionType.Exp, scale=0.2)
            nc.sync.dma_start(
                tb[b0 * 128 : (b0 + nb) * 128, :].rearrange(
                    "(b p) c -> p b c", p=128),
                ev[:, 0:nb, :])
    nc.compile()
    return nc


def _build_agg(cfg, C, layer):
    H, FT, ELEM, AOFF, A2OFF, TAILN, KW = (cfg["H"], cfg["FT"], cfg["ELEM"],
                                           cfg["AOFF"], cfg["A2OFF"],
                                           cfg["TAILN"], cfg["KW"])
    WOUT, HN = cfg["WOUT"], cfg["HN"]
    Cmax = int(C.max())
    seg_off, tot_slots, plan = _layout(C)
    COLS = tot_slots * 8

    nc = bacc.Bacc("TRN2", target_bir_lowering=False, debug=False, num_devices=CORES)
    T = nc.dram_tensor("T", [NW * WIN, ELEM], F16, kind="ExternalInput")
    idx_d = nc.dram_tensor("idx", [128, COLS], I16, kind="ExternalInput")
    ad_d = nc.dram_tensor("ad", [128, NB * H], F16, kind="ExternalInput")
    ident_d = nc.dram_tensor("ident", [128, 128], F16, kind="ExternalInput")
    if layer < 3:
        wt_d = nc.dram_tensor("wt", [KW, TAILN], F16, kind="ExternalInput")
        out_d = nc.dram_tensor("out", [SHP, TAILN], F16, kind="ExternalOutput")
    else:
        out_d = nc.dram_tensor("out", [SHP, 10], F32, kind="ExternalOutput")

    with tile.TileContext(nc) as tc, ExitStack() as ctx:
        cpool = ctx.enter_context(tc.tile_pool(name="c", bufs=1))
        gpool = ctx.enter_context(tc.tile_pool(name="g", bufs=3))
        apool = ctx.enter_context(tc.tile_pool(name="a", bufs=2))
        epool = ctx.enter_context(tc.tile_pool(name="e", bufs=2))
        bp2 = ctx.enter_context(tc.tile_pool(name="bp2", bufs=2))
        bp1 = ctx.enter_context(tc.tile_pool(name="bp1", bufs=1))
        ppo = ctx.enter_context(tc.tile_pool(name="ppo", bufs=2, space="PSUM"))
        ppt = ctx.enter_context(tc.tile_pool(name="ppt", bufs=1, space="PSUM"))

        idx_sb = cpool.tile([128, COLS], I16)
        nc.sync.dma_start(idx_sb[:, :], idx_d[:, :])
        ad_sb = cpool.tile([128, NB * H], F16)
        nc.sync.dma_start(ad_sb[:, :], ad_d[:, :])
        ident = cpool.tile([128, 128], F16)
        nc.sync.dma_start(ident[:, :], ident_d[:, :])
        if layer < 3:
            wt_sb = cpool.tile([KW, TAILN], F16)
            nc.sync.dma_start(wt_sb[:, :], wt_d[:, :])
        else:
            ls_sb = cpool.tile([128, NB, 16], F32)
        # gamma = exp(-0.8 * ad)
        gd_sb = cpool.tile([128, NB * H], F16)
        nc.scalar.activation(gd_sb[:, :], ad_sb[:, :],
                             mybir.ActivationFunctionType.Exp, scale=-0.8)
        s_acc = cpool.tile([128, NB, H], F32)
        nc.vector.memset(s_acc[:, :, :], 0.0)

        def epi_batch(groups, po_of):
            """Batched epilogue: normalize, LN stats via reduces (one Sqrt
            per batch), ELU, then per-block row build."""
            PB = 2 * len(groups)
            b0 = 2 * groups[0]
            add, sub, mult, mmin, mmax = (mybir.AluOpType.add, mybir.AluOpType.subtract,
                                          mybir.AluOpType.mult, mybir.AluOpType.min,
                                          mybir.AluOpType.max)
            rcpb = epool.tile([128, PB, H], F32, tag="rcpb")
            nc.vector.reciprocal(rcpb[:, :, :], s_acc[:, b0:b0 + PB, :])
            ponb = bp2.tile([128, PB, FT], F32, tag="ponb")
            for gi, g in enumerate(groups):
                nc.vector.tensor_tensor(
                    out=ponb[:, 2 * gi:2 * gi + 2, :].rearrange(
                        "p b (h r) -> p b h r", h=H),
                    in0=po_of[g][:, :].rearrange("p (b h r) -> p b h r", b=2, h=H),
                    in1=rcpb[:, 2 * gi:2 * gi + 2, :].to_broadcast(
                        [128, 2, H, FT // H]),
                    op=mult)
            if layer == 3:
                nc.vector.tensor_copy(ls_sb[:, b0:b0 + PB, :], ponb[:, :, :])
                return
            # LN statistics: mean/var via reduces (f32)
            mu = epool.tile([128, PB], F32, tag="mu")
            nc.vector.tensor_reduce(mu[:, :], ponb[:, :, :],
                                    axis=mybir.AxisListType.X, op=add)
            nc.vector.tensor_scalar(out=mu[:, :], in0=mu[:, :], scalar1=1.0 / FT,
                                    scalar2=None, op0=mult)
            sq = bp1.tile([128, PB, FT], F32, tag="sq")
            nc.vector.tensor_tensor(out=sq[:, :, :], in0=ponb[:, :, :],
                                    in1=ponb[:, :, :], op=mult)
            ex2 = epool.tile([128, PB], F32, tag="ex2")
            nc.vector.tensor_reduce(ex2[:, :], sq[:, :, :],
                                    axis=mybir.AxisListType.X, op=add)
            nc.vector.tensor_scalar(out=ex2[:, :], in0=ex2[:, :], scalar1=1.0 / FT,
                                    scalar2=None, op0=mult)
            mu2 = epool.tile([128, PB], F32, tag="mu2")
            nc.vector.tensor_tensor(out=mu2[:, :], in0=mu[:, :], in1=mu[:, :],
                                    op=mult)
            var = epool.tile([128, PB], F32, tag="var")
            nc.vector.tensor_tensor(out=var[:, :], in0=ex2[:, :], in1=mu2[:, :],
                                    op=sub)
            nc.vector.tensor_scalar(out=var[:, :], in0=var[:, :], scalar1=EPS,
                                    scalar2=None, op0=add)
            sd = epool.tile([128, PB], F32, tag="sd")
            nc.scalar.activation(sd[:, :], var[:, :],
                                 mybir.ActivationFunctionType.Sqrt)
            rstd = epool.tile([128, PB], F32, tag="rstd")
            nc.vector.reciprocal(rstd[:, :], sd[:, :])
            # y in place: (ponb - mu) * rstd
            nc.vector.tensor_tensor(out=ponb[:, :, :], in0=ponb[:, :, :],
                                    in1=mu[:, :].to_broadcast([128, PB, FT]),
                                    op=sub)
            nc.vector.tensor_tensor(out=ponb[:, :, :], in0=ponb[:, :, :],
                                    in1=rstd[:, :].to_broadcast([128, PB, FT]),
                                    op=mult)
            # ELU: max(y, exp(min(y, 0)) - 1)
            ym = bp1.tile([128, PB, FT], F32, tag="ym")
            nc.vector.tensor_scalar(out=ym[:, :, :], in0=ponb[:, :, :], scalar1=0.0,
                                    scalar2=None, op0=mmin)
            ee = bp1.tile([128, PB, FT], F32, tag="sq")
            nc.scalar.activation(ee[:, :, :], ym[:, :, :],
                                 mybir.ActivationFunctionType.Exp)
            nc.vector.tensor_scalar(out=ee[:, :, :], in0=ee[:, :, :], scalar1=1.0,
                                    scalar2=None, op0=sub)
            actb = bp1.tile([128, PB, FT], F16, tag="actb")
            nc.vector.tensor_tensor(out=actb[:, :, :], in0=ponb[:, :, :],
                                    in1=ee[:, :, :], op=mmax)
            for j in range(PB):
                bb = b0 + j
                pt = ppt.tile([FT, 128], F16, tag="pt")
                nc.tensor.transpose(pt[:, :], actb[:, j, :], ident[:, :])
                at = epool.tile([FT, 128], F16, tag="at")
                nc.vector.tensor_copy(at[:, :], pt[:, :])
                p2 = ppt.tile([128, TAILN], F32, tag="p2")
                nc.tensor.matmul(p2[:, :], at[:, :], wt_sb[:, :], start=True, stop=True)
                ev = epool.tile([128, TAILN], F16, tag="ev")
                nc.vector.tensor_copy(ev[:, :], p2[:, :])
                nc.scalar.activation(ev[:, WOUT:WOUT + HN], p2[:, WOUT:WOUT + HN],
                                     mybir.ActivationFunctionType.Exp)
                nc.scalar.activation(ev[:, WOUT + HN:WOUT + 2 * HN],
                                     p2[:, WOUT + HN:WOUT + 2 * HN],
                                     mybir.ActivationFunctionType.Exp, scale=0.2)
                nc.sync.dma_start(out_d[bb * 128 : (bb + 1) * 128, :], ev[:, :])

        for groups, wplans in plan:
            po_of = {}
            for gi, g in enumerate(groups):
                po_of[g] = ppo.tile([128, 2 * FT], F32, tag=f"po{gi}", name=f"po{gi}")
            for w in range(NW):
                for (cstart, segs) in wplans[w]:
                    S_sc = segs[-1][1] + 2 * segs[-1][2]
                    Gt = gpool.tile([128, SCAP, ELEM], F16, tag="G")
                    for a0 in range(0, S_sc, GMAX):
                        ns = min(GMAX, S_sc - a0) * 128
                        nc.gpsimd.dma_gather(
                            out_ap=Gt[:, a0:a0 + ns // 128, :],
                            in_ap=T[w * WIN:(w + 1) * WIN, :],
                            idxs_ap=idx_sb[:, (cstart + a0) * 8:
                                           (cstart + a0) * 8 + ns // 16],
                            num_idxs=ns, num_idxs_reg=ns, elem_size=ELEM)
                    for (g, off, Cg) in segs:
                        Av = Gt[:, off:off + 2 * Cg, AOFF:AOFF + H] \
                            .rearrange("p (b c) h -> p b c h", b=2)
                        A2v = Gt[:, off:off + 2 * Cg, A2OFF:A2OFF + H] \
                            .rearrange("p (b c) h -> p b c h", b=2)
                        gv = gd_sb[:, 2 * g * H:(2 * g + 2) * H] \
                            .rearrange("p (b h) -> p b h", b=2) \
                            .to_broadcast([128, 2, H, Cg]) \
                            .rearrange("p b h c -> p b c h")
                        m2 = apool.tile([128, 2, Cmax, H], F16, tag="m2")
                        nc.vector.tensor_tensor(out=m2[:, :, :Cg, :], in0=A2v,
                                                in1=gv, op=mybir.AluOpType.mult)
                        al = apool.tile([128, 2, Cmax, H], F16, tag="al")
                        nc.vector.tensor_tensor(out=al[:, :, :Cg, :], in0=Av,
                                                in1=m2[:, :, :Cg, :],
                                                op=mybir.AluOpType.max)
                        s_part = apool.tile([128, 2, H], F32, tag="s_part")
                        nc.vector.tensor_reduce(
                            s_part[:, :, :],
                            al[:, :, :Cg, :].rearrange("p b c h -> p b h c"),
                            axis=mybir.AxisListType.X, op=mybir.AluOpType.add)
                        nc.vector.tensor_tensor(out=s_acc[:, 2 * g:2 * g + 2, :],
                                                in0=s_acc[:, 2 * g:2 * g + 2, :],
                                                in1=s_part[:, :, :],
                                                op=mybir.AluOpType.add)
                        ax = apool.tile([128, Cmax, 2, FT], F16, tag="ax")
                        Gp = apool.tile([128, Cmax, 2, FT], F16, tag="Gp")
                        for b in range(2):
                            axv = ax[:, :Cg, b, :].rearrange(
                                "p c (h r) -> p c h r", h=H)
                            alv = al[:, b, :Cg, :].to_broadcast(
                                [128, Cg, H, FT // H])
                            nc.scalar.activation(
                                axv, alv, mybir.ActivationFunctionType.Copy)
                            nc.vector.tensor_tensor(
                                out=Gp[:, :Cg, b, :],
                                in0=Gt[:, off + b * Cg:off + (b + 1) * Cg, 0:FT],
                                in1=ax[:, :Cg, b, :],
                                op=mybir.AluOpType.mult)
                        po = po_of[g]
                        for cc in range(Cg):
                            nc.tensor.matmul(
                                po[:, :], ident[:, :],
                                Gp[:, cc, :, :].rearrange("p b f -> p (b f)"),
                                start=(w == 0 and cc == 0),
                                stop=(w == NW - 1 and cc == Cg - 1))
            epi_batch(groups, po_of)

        if layer == 3:
            rmax = cpool.tile([128, NB], F32)
            nc.vector.tensor_reduce(rmax[:, :], ls_sb[:, :, 0:10], axis=mybir.AxisListType.X,
                                    op=mybir.AluOpType.max)
            tt = cpool.tile([128, NB, 10], F32)
            nc.vector.tensor_tensor(out=tt[:, :, :], in0=ls_sb[:, :, 0:10],
                                    in1=rmax[:, :].to_broadcast([128, NB, 10]),
                                    op=mybir.AluOpType.subtract)
            ex = cpool.tile([128, NB, 10], F32)
            nc.scalar.activation(ex[:, :, :], tt[:, :, :], mybir.ActivationFunctionType.Exp)
            ssum = cpool.tile([128, NB], F32)
            nc.vector.tensor_reduce(ssum[:, :], ex[:, :, :], axis=mybir.AxisListType.X,
                                    op=mybir.AluOpType.add)
            lg = cpool.tile([128, NB], F32)
            nc.scalar.activation(lg[:, :], ssum[:, :], mybir.ActivationFunctionType.Ln)
            fin = cpool.tile([128, NB, 10], F32)
            nc.vector.tensor_tensor(out=fin[:, :, :], in0=tt[:, :, :],
                                    in1=lg[:, :].to_broadcast([128, NB, 10]),
                                    op=mybir.AluOpType.subtract)
            nc.sync.dma_start(out_d.ap().rearrange("(b p) c -> p b c", p=128), fin[:, :, :])
    nc.compile()
    return nc


def _ad_layout(tbl, H):
    """[SHP, H] (perm order) -> [128, NB*H] with [p, b*H+k] = tbl[b*128+p, k]."""
    return np.ascontiguousarray(tbl.reshape(NB, 128, H).transpose(1, 0, 2).reshape(128, NB * H))


LAST_EXEC_NS = 0
_TRACE_OK = None


def _trace_available():
    global _TRACE_OK
    if _TRACE_OK is None:
        try:
            from antenv.axon_hooks import get_axon_ntff_profile_hook
            _TRACE_OK = get_axon_ntff_profile_hook() is not None
        except Exception:
            _TRACE_OK = False
    return _TRACE_OK


def _hw_runner(nc, in_maps):
    global LAST_EXEC_NS
    try:
        r = run_bass_kernel_spmd(nc, in_maps, core_ids=list(range(CORES)),
                                 trace=_trace_available())
        if r.exec_time_ns:
            LAST_EXEC_NS += r.exec_time_ns
    except Exception:
        r = run_bass_kernel_spmd(nc, in_maps, core_ids=list(range(CORES)), trace=False)
    return r.results


def _scatter_table(rows_by_core, pc, row_map, elem):
    """Assemble global table [NW*WIN, elem] from per-core shard rows
    (perm order); duplicated rows are written at every window copy.
    Pad rows stay all-zero (zero attention weight)."""
    rows_global, rows_node = row_map
    T = np.zeros((NW * WIN, elem), np.float16)
    ncols = rows_by_core[0].shape[1]
    full = np.zeros((N, ncols), np.float16)
    for c in range(CORES):
        pos_node = pc[c]["pos_node"]
        real = pos_node >= 0
        full[pos_node[real]] = rows_by_core[c][real]
    T[rows_global, 0:ncols] = full[rows_node]
    return T


def kernel(_runner=None, **inputs):
    runner = _runner or _hw_runner
    x = np.asarray(inputs["x"])
    edge_index = np.asarray(inputs["edge_index"])
    pc, C, row_map, tot_slots = _host_prep(edge_index)
    wt1, wt2, wt3 = _wt_tables(inputs)
    ident = np.eye(128, dtype=np.float16)

    # ---- launch 0: h1 | exp(as1) | exp(.2 as1) | ad1 shard build ----
    nc0 = _build_launch0()
    in_maps0 = []
    for c in range(CORES):
        pos_node = pc[c]["pos_node"]
        xs = np.zeros((SHP, 128), np.float16)
        real = pos_node >= 0
        xs[real] = x[pos_node[real]].astype(np.float16)
        in_maps0.append({"x_t": np.ascontiguousarray(xs.T), "wt1": wt1})
    r0 = runner(nc0, in_maps0)
    tb = [r0[c]["tb"] for c in range(CORES)]

    T = _scatter_table([t[:, 0:144] for t in tb], pc, row_map, L1["ELEM"])
    ads = [_ad_layout(tb[c][:, 144:152], 8) for c in range(CORES)]

    for layer, cfg in ((1, L1), (2, L2), (3, L3)):
        nc = _build_agg(cfg, C, layer)
        in_maps = []
        for c in range(CORES):
            m = {"T": T, "idx": pc[c]["idx"], "ad": ads[c], "ident": ident}
            if layer == 1:
                m["wt"] = wt2
            elif layer == 2:
                m["wt"] = wt3
            in_maps.append(m)
        r = runner(nc, in_maps)
        outs = [r[c]["out"] for c in range(CORES)]
        if layer == 1:
            # ev rows [h2(64) | A(4) | A2(4) | ad2(4)]
            T = _scatter_table([o[:, 0:72] for o in outs], pc, row_map, L2["ELEM"])
            ads = [_ad_layout(outs[c][:, 72:76], 4) for c in range(CORES)]
        elif layer == 2:
            # ev rows [h3(10) | A(1) | A2(1) | ad3(1) | pad]
            T = _scatter_table([o[:, 0:12] for o in outs], pc, row_map, L3["ELEM"])
            ads = [_ad_layout(outs[c][:, 12:13], 1) for c in range(CORES)]

    result = np.empty((N, 10), np.float32)
    for c in range(CORES):
        pos_node = pc[c]["pos_node"]
        real = pos_node >= 0
        result[pos_node[real]] = outs[c][real]
    return result
